# revision 4
# baseline (speedup 1.0000x reference)
"""Trainium2 Bass kernel for nn_CubicalModel_ISM.

Reference computation:
    Xp = reshape(I1 @ p0, (28, 28)); Yp = reshape(I2 @ p1, (28, 28))
    dgm1 = Xp[inds1[0::2], inds1[1::2]].reshape(50, 2)
    dgm2 = Yp[inds2[0::2], inds2[1::2]].reshape(50, 2)

Only the <=100 gathered rows of each 784-row GEMV are live, and the gather
commutes with the per-row dot product.  So the host selects the 100 indexed
rows of I1 and of I2 (the "tiny gather", applied to the input instead of the
output), the device computes the 200 surviving dot products of length 32768
with k sharded over the 8 cores, and the host sums the partial vectors.

The DMA phase is per-SDMA-engine bandwidth-bound (16 engines x ~26.3 GB/s
= 421 GB/s/core aggregate), so bytes are everything.  3-byte encoding:
    A ~= Ahi(fp16) + Alo8(e4m3)/4096
Alo = A - Ahi is ~2^-11*|A|, far below e4m3's 2^-9 subnormal floor, so it
is pre-scaled by 2^12 (range ~[0.004, 8], all e4m3-normal).  The scaled
term accumulates in its OWN PSUM banks and the host divides by 4096.
    A.q ~= Ahi.qhi + Ahi.qlo          (fp16 matmuls, ps_a/ps_b)
         + Alo8.(q8a + q8b) / 4096    (fp8 matmuls,  ps3_a/ps3_b)
where q8a = e4m3(qhi), q8b = e4m3(qhi - q8a) (two fp8 cols kill the fp8
q-quantisation error).  Measured max elementwise rel err 5.6e-4 on the
seed-0 inputs (gate 2e-2); fp16-pair baseline was 9.5e-5.

Per-core program: 4096 k-rows as 8 logical tiles of four 128-row k-chunks.
Tile byte layout ([128 partitions, 2448 B], tile dtype f8e4; fp16 regions
read through AP.bitcast):
    [   0:1600)  hi fp16: c0,c1,c2,c3 each 200 cols (I1 100 | I2 100)
    [1600:1632)  q fp16: qhi pair0 | qlo pair0 | qhi pair1 | qlo pair1
    [1632:2432)  lo fp8 (x4096): c0..c3 each 200 cols
    [2432:2448)  q8: pair0 (q8a c0,c1 | q8b c0,c1) | pair1
Four matmuls per tile: fp16 (qhi+qlo)x(hi pair) into ps_a/ps_b, fp8
(q8a+q8b)x(lo pair) into ps3_a/ps3_b.  Rows {0,4}/{2,6} x cols [0:100)/
[200:300) hold dgm1 partials, rows {1,5}/{3,7} x [100:200)/[300:400) dgm2;
off-segment entries are garbage cross terms, never read.

DMA: 9 pieces (tiles t0..t6 whole at 306 KB, t7 split hi 204 KB + lo
102 KB) alternating the SP/ACT HWDGE queues; the small final pieces keep
the post-last-byte matmul tail short.  PE_HAM warmup matmuls spin while
the first DMAs are in flight so the real matmuls run at 2.4 GHz.
Output: ps banks copied to SBUF by DVE (a-banks) and ACT (b-banks) in
parallel, stored as two halves so the first store's latency hides under
the second copy.
"""

import numpy as np
import ml_dtypes

K = 32768
NCORES = 8
KS = K // NCORES          # 4096 k columns per core
T = KS // 512             # 8 logical tiles of 4 k-chunks
WB = 2448                 # tile width in bytes (fp8 cols)
WH = 1632                 # hi+q fp16 region bytes
WL = 816                  # lo+q8 fp8 region bytes
R = 100                   # gathered rows per diagram
SIDE = 28
LOSCALE = 4096.0          # Alo pre-scale (2^12)

F8 = ml_dtypes.float8_e4m3

_cache = {}


def _build_nc():
    import concourse.bacc as bacc
    import concourse.mybir as mybir
    from concourse.tile import TileContext

    f32 = mybir.dt.float32
    f16 = mybir.dt.float16
    f8 = mybir.dt.float8e4
    nc = bacc.Bacc("TRN2", target_bir_lowering=False, debug=False,
                   num_devices=NCORES)

    # one DRAM param per DMA piece
    a_full = [nc.declare_dram_parameter(f"a{t}", [128, WB], f8, isOutput=False)
              for t in range(T - 1)]
    a7h = nc.declare_dram_parameter("a7h", [128, WH], f8, isOutput=False)
    a7l = nc.declare_dram_parameter("a7l", [128, WL], f8, isOutput=False)
    y = nc.declare_dram_parameter("y", [8, 1600], f32, isOutput=True)

    with TileContext(nc) as tc:
        with (
            tc.tile_pool(name="apool", bufs=T - 1) as apool,
            tc.tile_pool(name="hpool", bufs=1) as hpool,
            tc.tile_pool(name="lpool", bufs=1) as lpool,
            tc.tile_pool(name="wpool", bufs=1) as wpool,
            tc.tile_pool(name="opool", bufs=1) as opool,
            tc.tile_pool(name="ps", bufs=1, space="PSUM") as pspool,
        ):
            # full-partition tiles so each lands at PSUM base partition 0
            ps_a = pspool.tile([128, 400], f32, name="ps_a", tag="ps_a")[0:8, :]
            ps_b = pspool.tile([128, 400], f32, name="ps_b", tag="ps_b")[0:8, :]
            ps3_a = pspool.tile([128, 400], f32, name="ps3_a", tag="ps3_a")[0:8, :]
            ps3_b = pspool.tile([128, 400], f32, name="ps3_b", tag="ps3_b")[0:8, :]

            # PE_HAM releases the clock gate (1.2 -> 2.4 GHz) only after
            # ~3.4 us of sustained PE activity.  Spin dummy matmuls on a
            # zeroed tile while the first DMAs are in flight so the real
            # matmuls run warm.
            warm = wpool.tile([128, 512], f16, name="warm", tag="warm")
            nc.gpsimd.memset(warm, 0.0)
            ps_w = pspool.tile([128, 512], f32, name="ps_w", tag="ps_w")[0:8, :]

            # issue every DMA up front; descriptors queue in the two HWDGE
            # rings and drain back-to-back at the 16-engine aggregate rate
            tiles = []
            for t in range(T - 1):
                at = apool.tile([128, WB], f8)
                eng = nc.sync if t % 2 == 0 else nc.scalar
                eng.dma_start(out=at, in_=a_full[t][:])
                tiles.append(at)
            t7h = hpool.tile([128, WH], f8)
            nc.scalar.dma_start(out=t7h, in_=a7h[:])
            t7l = lpool.tile([128, WL], f8)
            nc.scalar.dma_start(out=t7l, in_=a7l[:])

            for _ in range(10):
                nc.tensor.matmul(ps_w, warm[:, 0:8], warm[:, 0:512],
                                 start=True, stop=True)

            def hi_mms(src, t, off=0):
                first = t == 0
                last = t == T - 1
                nc.tensor.matmul(
                    ps_a, src[:, off + 1600:off + 1616].bitcast(f16),
                    src[:, off + 0:off + 800].bitcast(f16),
                    start=first, stop=last)
                nc.tensor.matmul(
                    ps_b, src[:, off + 1616:off + 1632].bitcast(f16),
                    src[:, off + 800:off + 1600].bitcast(f16),
                    start=first, stop=last)

            def lo_mms(src, t, off=0):
                first = t == 0
                last = t == T - 1
                nc.tensor.matmul(
                    ps3_a, src[:, off + 800:off + 808],
                    src[:, off + 0:off + 400],
                    start=first, stop=last)
                nc.tensor.matmul(
                    ps3_b, src[:, off + 808:off + 816],
                    src[:, off + 400:off + 800],
                    start=first, stop=last)

            for t in range(T - 1):
                hi_mms(tiles[t], t)
                lo_mms(tiles[t], t, off=WH)
            hi_mms(t7h, T - 1)
            lo_mms(t7l, T - 1)

            yt = opool.tile([8, 1600], f32)
            nc.vector.tensor_copy(out=yt[:, 0:400], in_=ps_a)
            nc.vector.tensor_copy(out=yt[:, 400:800], in_=ps3_a)
            nc.scalar.copy(out=yt[:, 800:1200], in_=ps_b)
            nc.scalar.copy(out=yt[:, 1200:1600], in_=ps3_b)
            nc.sync.dma_start(out=y[:, 0:800], in_=yt[:, 0:800])
            nc.sync.dma_start(out=y[:, 800:1600], in_=yt[:, 800:1600])
    nc.compile()
    return nc


def _split16(x):
    hi = x.astype(np.float16)
    lo = (x - hi.astype(np.float32))
    return hi, lo


def _prep_inputs(p0, p1, I1, I2, inds1, inds2):
    idx1 = inds1.astype(np.int64).reshape(-1, 2)
    idx2 = inds2.astype(np.int64).reshape(-1, 2)
    rows1 = idx1[:, 0] * SIDE + idx1[:, 1]      # flat positions, in order
    rows2 = idx2[:, 0] * SIDE + idx2[:, 1]

    selT = np.empty((K, 2 * R), np.float32)
    selT[:, 0:R] = I1[rows1, :].T
    selT[:, R:2 * R] = I2[rows2, :].T
    sel_hi, sel_lo = _split16(selT)             # fp16 [K,200], fp32 resid
    sel_lo8 = (sel_lo * LOSCALE).astype(F8)     # scaled e4m3 [K,200]
    q = np.stack([p0, p1], axis=-1)             # [K, 2]
    q_hi, _qlo = _split16(q)
    q_lo = _qlo.astype(np.float16)
    q8a = q_hi.astype(np.float32).astype(F8)
    q8b = (q_hi.astype(np.float32) - q8a.astype(np.float32)).astype(F8)

    in_maps = []
    for cix in range(NCORES):
        o = cix * KS
        bh = sel_hi[o:o + KS].reshape(T, 4, 128, 2 * R)
        bl = sel_lo8[o:o + KS].reshape(T, 4, 128, 2 * R)
        qh = q_hi[o:o + KS].reshape(T, 2, 2, 128, 2)
        ql = q_lo[o:o + KS].reshape(T, 2, 2, 128, 2)
        qa = q8a[o:o + KS].reshape(T, 2, 2, 128, 2)
        qb = q8b[o:o + KS].reshape(T, 2, 2, 128, 2)

        a16 = np.empty((T, 128, 816), np.float16)   # hi + q fp16 region
        for c in range(4):
            a16[:, :, c * 200:(c + 1) * 200] = bh[:, c]
        a16[:, :, 800:802] = qh[:, 0, 0]
        a16[:, :, 802:804] = qh[:, 0, 1]
        a16[:, :, 804:806] = ql[:, 0, 0]
        a16[:, :, 806:808] = ql[:, 0, 1]
        a16[:, :, 808:810] = qh[:, 1, 0]
        a16[:, :, 810:812] = qh[:, 1, 1]
        a16[:, :, 812:814] = ql[:, 1, 0]
        a16[:, :, 814:816] = ql[:, 1, 1]

        a8 = np.empty((T, 128, 816), F8)            # lo + q8 fp8 region
        for c in range(4):
            a8[:, :, c * 200:(c + 1) * 200] = bl[:, c]
        a8[:, :, 800:802] = qa[:, 0, 0]
        a8[:, :, 802:804] = qa[:, 0, 1]
        a8[:, :, 804:806] = qb[:, 0, 0]
        a8[:, :, 806:808] = qb[:, 0, 1]
        a8[:, :, 808:810] = qa[:, 1, 0]
        a8[:, :, 810:812] = qa[:, 1, 1]
        a8[:, :, 812:814] = qb[:, 1, 0]
        a8[:, :, 814:816] = qb[:, 1, 1]

        ab = np.empty((T, 128, WB), np.uint8)
        ab[:, :, 0:WH] = a16.view(np.uint8).reshape(T, 128, WH)
        ab[:, :, WH:WB] = a8.view(np.uint8)

        m = {f"a{t}": ab[t].view(F8) for t in range(T - 1)}
        m["a7h"] = ab[T - 1, :, 0:WH].copy().view(F8)
        m["a7l"] = ab[T - 1, :, WH:WB].copy().view(F8)
        in_maps.append(m)
    return in_maps


def _run(in_maps, trace=False):
    from concourse.bass_utils import run_bass_kernel_spmd

    if "nc" not in _cache:
        _cache["nc"] = _build_nc()
    return run_bass_kernel_spmd(
        _cache["nc"], in_maps, list(range(NCORES)), trace=trace
    )


def kernel(p0, p1, I1, I2, inds1, inds2):
    p0 = np.ascontiguousarray(np.asarray(p0, dtype=np.float32))
    p1 = np.ascontiguousarray(np.asarray(p1, dtype=np.float32))
    I1 = np.asarray(I1, dtype=np.float32)
    I2 = np.asarray(I2, dtype=np.float32)
    inds1 = np.asarray(inds1)
    inds2 = np.asarray(inds2)

    in_maps = _prep_inputs(p0, p1, I1, I2, inds1, inds2)
    results = _run(in_maps).results

    acc = np.zeros((2, R), np.float64)
    for r in results:
        yf = r["y"].astype(np.float64)
        for base, sc in ((0, 1.0), (400, 1.0 / LOSCALE),
                         (800, 1.0), (1200, 1.0 / LOSCALE)):
            yc = yf[:, base:base + 400]
            acc[0] += sc * (yc[0, 0:100] + yc[4, 0:100]
                            + yc[2, 200:300] + yc[6, 200:300])    # dgm1
            acc[1] += sc * (yc[1, 100:200] + yc[5, 100:200]
                            + yc[3, 300:400] + yc[7, 300:400])    # dgm2
    vals = acc.astype(np.float32)
    dgm1 = vals[0].reshape(R // 2, 2)
    dgm2 = vals[1].reshape(R // 2, 2)
    return (dgm1, dgm2)


# revision 6
# speedup vs baseline: 1.0774x; 1.0774x over previous
"""Trainium2 Bass kernel for nn_CubicalModel_ISM.

Reference computation:
    Xp = reshape(I1 @ p0, (28, 28)); Yp = reshape(I2 @ p1, (28, 28))
    dgm1 = Xp[inds1[0::2], inds1[1::2]].reshape(50, 2)
    dgm2 = Yp[inds2[0::2], inds2[1::2]].reshape(50, 2)

Only the <=100 gathered rows of each 784-row GEMV are live, and the gather
commutes with the per-row dot product.  So the host selects the 100 indexed
rows of I1 and of I2 (the "tiny gather", applied to the input instead of the
output), the device computes the 200 surviving dot products of length 32768
with k sharded over the 8 cores, and the host sums the partial vectors.

The DMA phase is per-SDMA-engine bandwidth-bound (16 engines x ~26.3 GB/s
= 421 GB/s/core aggregate), so bytes are everything.  3-byte encoding:
    A ~= Ahi(fp16) + Alo8(e4m3)/4096
Alo = A - Ahi is ~2^-11*|A|, far below e4m3's 2^-9 subnormal floor, so it
is pre-scaled by 2^12 (range ~[0.004, 8], all e4m3-normal).  The scaled
term accumulates in its OWN PSUM banks and the host divides by 4096.
    A.q ~= Ahi.qhi + Ahi.qlo          (fp16 matmuls, ps_a/ps_b)
         + Alo8.(q8a + q8b) / 4096    (fp8 matmuls,  ps3_a/ps3_b)
where q8a = e4m3(qhi), q8b = e4m3(qhi - q8a) (two fp8 cols kill the fp8
q-quantisation error).  Measured max elementwise rel err 5.6e-4 on the
seed-0 inputs (gate 2e-2); fp16-pair baseline was 9.5e-5.

Per-core program: 4096 k-rows as 8 logical tiles of four 128-row k-chunks.
Tile byte layout ([128 partitions, 2448 B], tile dtype f8e4; fp16 regions
read through AP.bitcast):
    [   0:1600)  hi fp16: c0,c1,c2,c3 each 200 cols (I1 100 | I2 100)
    [1600:1632)  q fp16: qhi pair0 | qlo pair0 | qhi pair1 | qlo pair1
    [1632:2432)  lo fp8 (x4096): c0..c3 each 200 cols
    [2432:2448)  q8: pair0 (q8a c0,c1 | q8b c0,c1) | pair1
Four matmuls per tile: fp16 (qhi+qlo)x(hi pair) into ps_a/ps_b, fp8
(q8a+q8b)x(lo pair) into ps3_a/ps3_b.  Rows {0,4}/{2,6} x cols [0:100)/
[200:300) hold dgm1 partials, rows {1,5}/{3,7} x [100:200)/[300:400) dgm2;
off-segment entries are garbage cross terms, never read.

DMA: 9 pieces (tiles t0..t6 whole at 306 KB, t7 split hi 204 KB + lo
102 KB) alternating the SP/ACT HWDGE queues; the small final pieces keep
the post-last-byte matmul tail short.  PE_HAM warmup matmuls spin while
the first DMAs are in flight so the real matmuls run at 2.4 GHz.
Output: ps banks copied to SBUF by DVE (a-banks) and ACT (b-banks) in
parallel, stored as two halves so the first store's latency hides under
the second copy.
"""

import numpy as np
import ml_dtypes

K = 32768
NCORES = 8
KS = K // NCORES          # 4096 k columns per core
T = KS // 512             # 8 logical tiles of 4 k-chunks
WB = 2448                 # tile width in bytes (fp8 cols)
WH = 1632                 # hi+q fp16 region bytes
WL = 816                  # lo+q8 fp8 region bytes
R = 100                   # gathered rows per diagram
SIDE = 28
LOSCALE = 4096.0          # Alo pre-scale (2^12)

F8 = ml_dtypes.float8_e4m3

_cache = {}


def _build_nc():
    import concourse.bacc as bacc
    import concourse.mybir as mybir
    from concourse.tile import TileContext

    f32 = mybir.dt.float32
    f16 = mybir.dt.float16
    f8 = mybir.dt.float8e4
    nc = bacc.Bacc("TRN2", target_bir_lowering=False, debug=False,
                   num_devices=NCORES)

    # DMA pieces: SP gets t0 | t3t4 | t7, ACT gets t1t2 | t5t6 (1.22 MB
    # per queue).  Two tiles per mid-piece keeps the per-dma_start issue
    # cost (~0.7 us on the issuing engine) off the critical path while the
    # small first piece (t0) lets the matmuls start early and the small
    # last piece (t7) keeps the post-last-byte matmul tail short.
    A0 = nc.declare_dram_parameter("A0", [128, WB], f8, isOutput=False)
    A1 = nc.declare_dram_parameter("A1", [128, 2 * WB], f8, isOutput=False)
    A2 = nc.declare_dram_parameter("A2", [128, 2 * WB], f8, isOutput=False)
    A3 = nc.declare_dram_parameter("A3", [128, 2 * WB], f8, isOutput=False)
    A4 = nc.declare_dram_parameter("A4", [128, WB], f8, isOutput=False)
    y = nc.declare_dram_parameter("y", [8, 1600], f32, isOutput=True)

    with TileContext(nc) as tc:
        with (
            tc.tile_pool(name="apool", bufs=1) as apool,
            tc.tile_pool(name="wpool", bufs=1) as wpool,
            tc.tile_pool(name="opool", bufs=1) as opool,
            tc.tile_pool(name="ps", bufs=1, space="PSUM") as pspool,
        ):
            # full-partition tiles so each lands at PSUM base partition 0
            ps_a = pspool.tile([128, 400], f32, name="ps_a", tag="ps_a")[0:8, :]
            ps_b = pspool.tile([128, 400], f32, name="ps_b", tag="ps_b")[0:8, :]
            ps3_a = pspool.tile([128, 400], f32, name="ps3_a", tag="ps3_a")[0:8, :]
            ps3_b = pspool.tile([128, 400], f32, name="ps3_b", tag="ps3_b")[0:8, :]

            # PE_HAM releases the clock gate (1.2 -> 2.4 GHz) only after
            # ~3.4 us of sustained PE activity.  Spin dummy matmuls on a
            # zeroed tile while the first DMAs are in flight so the real
            # matmuls run warm.
            warm = wpool.tile([128, 512], f16, name="warm", tag="warm")
            nc.gpsimd.memset(warm, 0.0)
            ps_w = pspool.tile([128, 512], f32, name="ps_w", tag="ps_w")[0:8, :]

            Tb0 = apool.tile([128, WB], f8, name="Tb0", tag="Tb0")
            Tb1 = apool.tile([128, 2 * WB], f8, name="Tb1", tag="Tb1")
            Tb2 = apool.tile([128, 2 * WB], f8, name="Tb2", tag="Tb2")
            Tb3 = apool.tile([128, 2 * WB], f8, name="Tb3", tag="Tb3")
            Tb4 = apool.tile([128, WB], f8, name="Tb4", tag="Tb4")
            nc.sync.dma_start(out=Tb0, in_=A0[:])
            nc.scalar.dma_start(out=Tb1, in_=A1[:])
            nc.sync.dma_start(out=Tb2, in_=A2[:])
            nc.scalar.dma_start(out=Tb3, in_=A3[:])
            nc.sync.dma_start(out=Tb4, in_=A4[:])

            srcs = [Tb0[:, 0:WB],
                    Tb1[:, 0:WB], Tb1[:, WB:2 * WB],
                    Tb2[:, 0:WB], Tb2[:, WB:2 * WB],
                    Tb3[:, 0:WB], Tb3[:, WB:2 * WB],
                    Tb4[:, 0:WB]]

            for _ in range(10):
                nc.tensor.matmul(ps_w, warm[:, 0:8], warm[:, 0:512],
                                 start=True, stop=True)

            # Bank parity by tile: tiles t0..t5 accumulate into the a
            # banks (stop at t5 -> their copies and store overlap the b
            # tiles' matmuls), t6..t7 into the b banks.  Both chunk pairs
            # of a tile go to the same bank - the host sums every segment
            # partial anyway.  Intra-tile fp16/fp8 alternation keeps
            # consecutive matmuls on different PSUM banks so they pipeline.
            for t in range(T):
                src = srcs[t]
                ps, ps3 = (ps_a, ps3_a) if t < 6 else (ps_b, ps3_b)
                first = t == 0 or t == 6
                last = t == 5 or t == T - 1
                nc.tensor.matmul(
                    ps, src[:, 1600:1616].bitcast(f16),
                    src[:, 0:800].bitcast(f16), start=first, stop=False)
                nc.tensor.matmul(
                    ps3, src[:, WH + 800:WH + 808],
                    src[:, WH + 0:WH + 400], start=first, stop=False)
                nc.tensor.matmul(
                    ps, src[:, 1616:1632].bitcast(f16),
                    src[:, 800:1600].bitcast(f16), start=False, stop=last)
                nc.tensor.matmul(
                    ps3, src[:, WH + 808:WH + 816],
                    src[:, WH + 400:WH + 800], start=False, stop=last)

            yt = opool.tile([8, 1600], f32)
            nc.vector.tensor_copy(out=yt[:, 0:400], in_=ps_a)
            nc.scalar.copy(out=yt[:, 400:800], in_=ps3_a)
            nc.sync.dma_start(out=y[:, 0:800], in_=yt[:, 0:800])
            nc.vector.tensor_copy(out=yt[:, 800:1200], in_=ps_b)
            nc.scalar.copy(out=yt[:, 1200:1600], in_=ps3_b)
            nc.sync.dma_start(out=y[:, 800:1600], in_=yt[:, 800:1600])
    nc.compile()
    return nc


def _split16(x):
    hi = x.astype(np.float16)
    lo = (x - hi.astype(np.float32))
    return hi, lo


def _prep_inputs(p0, p1, I1, I2, inds1, inds2):
    idx1 = inds1.astype(np.int64).reshape(-1, 2)
    idx2 = inds2.astype(np.int64).reshape(-1, 2)
    rows1 = idx1[:, 0] * SIDE + idx1[:, 1]      # flat positions, in order
    rows2 = idx2[:, 0] * SIDE + idx2[:, 1]

    selT = np.empty((K, 2 * R), np.float32)
    selT[:, 0:R] = I1[rows1, :].T
    selT[:, R:2 * R] = I2[rows2, :].T
    sel_hi, sel_lo = _split16(selT)             # fp16 [K,200], fp32 resid
    sel_lo8 = (sel_lo * LOSCALE).astype(F8)     # scaled e4m3 [K,200]
    q = np.stack([p0, p1], axis=-1)             # [K, 2]
    q_hi, _qlo = _split16(q)
    q_lo = _qlo.astype(np.float16)
    q8a = q_hi.astype(np.float32).astype(F8)
    q8b = (q_hi.astype(np.float32) - q8a.astype(np.float32)).astype(F8)

    in_maps = []
    for cix in range(NCORES):
        o = cix * KS
        bh = sel_hi[o:o + KS].reshape(T, 4, 128, 2 * R)
        bl = sel_lo8[o:o + KS].reshape(T, 4, 128, 2 * R)
        qh = q_hi[o:o + KS].reshape(T, 2, 2, 128, 2)
        ql = q_lo[o:o + KS].reshape(T, 2, 2, 128, 2)
        qa = q8a[o:o + KS].reshape(T, 2, 2, 128, 2)
        qb = q8b[o:o + KS].reshape(T, 2, 2, 128, 2)

        a16 = np.empty((T, 128, 816), np.float16)   # hi + q fp16 region
        for c in range(4):
            a16[:, :, c * 200:(c + 1) * 200] = bh[:, c]
        a16[:, :, 800:802] = qh[:, 0, 0]
        a16[:, :, 802:804] = qh[:, 0, 1]
        a16[:, :, 804:806] = ql[:, 0, 0]
        a16[:, :, 806:808] = ql[:, 0, 1]
        a16[:, :, 808:810] = qh[:, 1, 0]
        a16[:, :, 810:812] = qh[:, 1, 1]
        a16[:, :, 812:814] = ql[:, 1, 0]
        a16[:, :, 814:816] = ql[:, 1, 1]

        a8 = np.empty((T, 128, 816), F8)            # lo + q8 fp8 region
        for c in range(4):
            a8[:, :, c * 200:(c + 1) * 200] = bl[:, c]
        a8[:, :, 800:802] = qa[:, 0, 0]
        a8[:, :, 802:804] = qa[:, 0, 1]
        a8[:, :, 804:806] = qb[:, 0, 0]
        a8[:, :, 806:808] = qb[:, 0, 1]
        a8[:, :, 808:810] = qa[:, 1, 0]
        a8[:, :, 810:812] = qa[:, 1, 1]
        a8[:, :, 812:814] = qb[:, 1, 0]
        a8[:, :, 814:816] = qb[:, 1, 1]

        ab = np.empty((T, 128, WB), np.uint8)
        ab[:, :, 0:WH] = a16.view(np.uint8).reshape(T, 128, WH)
        ab[:, :, WH:WB] = a8.view(np.uint8)

        def pair(u):
            return np.ascontiguousarray(
                ab[u:u + 2].transpose(1, 0, 2)).reshape(128, 2 * WB)

        m = {"A0": ab[0].view(F8), "A1": pair(1).view(F8),
             "A2": pair(3).view(F8), "A3": pair(5).view(F8),
             "A4": ab[7].view(F8)}
        in_maps.append(m)
    return in_maps


def _run(in_maps, trace=False):
    from concourse.bass_utils import run_bass_kernel_spmd

    if "nc" not in _cache:
        _cache["nc"] = _build_nc()
    return run_bass_kernel_spmd(
        _cache["nc"], in_maps, list(range(NCORES)), trace=trace
    )


def kernel(p0, p1, I1, I2, inds1, inds2):
    p0 = np.ascontiguousarray(np.asarray(p0, dtype=np.float32))
    p1 = np.ascontiguousarray(np.asarray(p1, dtype=np.float32))
    I1 = np.asarray(I1, dtype=np.float32)
    I2 = np.asarray(I2, dtype=np.float32)
    inds1 = np.asarray(inds1)
    inds2 = np.asarray(inds2)

    in_maps = _prep_inputs(p0, p1, I1, I2, inds1, inds2)
    results = _run(in_maps).results

    acc = np.zeros((2, R), np.float64)
    for r in results:
        yf = r["y"].astype(np.float64)
        for base, sc in ((0, 1.0), (400, 1.0 / LOSCALE),
                         (800, 1.0), (1200, 1.0 / LOSCALE)):
            yc = yf[:, base:base + 400]
            acc[0] += sc * (yc[0, 0:100] + yc[4, 0:100]
                            + yc[2, 200:300] + yc[6, 200:300])    # dgm1
            acc[1] += sc * (yc[1, 100:200] + yc[5, 100:200]
                            + yc[3, 300:400] + yc[7, 300:400])    # dgm2
    vals = acc.astype(np.float32)
    dgm1 = vals[0].reshape(R // 2, 2)
    dgm2 = vals[1].reshape(R // 2, 2)
    return (dgm1, dgm2)


# revision 8
# speedup vs baseline: 1.0983x; 1.0193x over previous
"""Trainium2 Bass kernel for nn_CubicalModel_ISM.

Reference computation:
    Xp = reshape(I1 @ p0, (28, 28)); Yp = reshape(I2 @ p1, (28, 28))
    dgm1 = Xp[inds1[0::2], inds1[1::2]].reshape(50, 2)
    dgm2 = Yp[inds2[0::2], inds2[1::2]].reshape(50, 2)

Only the <=100 gathered rows of each 784-row GEMV are live, and the gather
commutes with the per-row dot product.  So the host selects the 100 indexed
rows of I1 and of I2 (the "tiny gather", applied to the input instead of the
output), the device computes the 200 surviving dot products of length 32768
with k sharded over the 8 cores, and the host sums the partial vectors.

The DMA phase is per-SDMA-engine bandwidth-bound (16 engines x ~26.3 GB/s
= 421 GB/s/core aggregate), so bytes are everything.  3-byte encoding:
    A ~= Ahi(fp16) + Alo8(e4m3)/4096
Alo = A - Ahi is ~2^-11*|A|, far below e4m3's 2^-9 subnormal floor, so it
is pre-scaled by 2^12 (range ~[0.004, 8], all e4m3-normal).  The scaled
term accumulates in its OWN PSUM banks and the host divides by 4096.
    A.q ~= Ahi.qhi + Ahi.qlo          (fp16 matmuls, ps_a/ps_b)
         + Alo8.(q8a + q8b) / 4096    (fp8 matmuls,  ps3_a/ps3_b)
where q8a = e4m3(qhi), q8b = e4m3(qhi - q8a) (two fp8 cols kill the fp8
q-quantisation error).  Measured max elementwise rel err 5.6e-4 on the
seed-0 inputs (gate 2e-2); fp16-pair baseline was 9.5e-5.

Per-core program: 4096 k-rows as 8 logical tiles of four 128-row k-chunks.
Tile byte layout ([128 partitions, 2448 B], tile dtype f8e4; fp16 regions
read through AP.bitcast):
    [   0:1600)  hi fp16: c0,c1,c2,c3 each 200 cols (I1 100 | I2 100)
    [1600:1632)  q fp16: qhi pair0 | qlo pair0 | qhi pair1 | qlo pair1
    [1632:2432)  lo fp8 (x4096): c0..c3 each 200 cols
    [2432:2448)  q8: pair0 (q8a c0,c1 | q8b c0,c1) | pair1
Four matmuls per tile: fp16 (qhi+qlo)x(hi pair) into ps_a/ps_b, fp8
(q8a+q8b)x(lo pair) into ps3_a/ps3_b.  Rows {0,4}/{2,6} x cols [0:100)/
[200:300) hold dgm1 partials, rows {1,5}/{3,7} x [100:200)/[300:400) dgm2;
off-segment entries are garbage cross terms, never read.

DMA: 9 pieces (tiles t0..t6 whole at 306 KB, t7 split hi 204 KB + lo
102 KB) alternating the SP/ACT HWDGE queues; the small final pieces keep
the post-last-byte matmul tail short.  PE_HAM warmup matmuls spin while
the first DMAs are in flight so the real matmuls run at 2.4 GHz.
Output: ps banks copied to SBUF by DVE (a-banks) and ACT (b-banks) in
parallel, stored as two halves so the first store's latency hides under
the second copy.
"""

import numpy as np
import ml_dtypes

K = 32768
NCORES = 8
KS = K // NCORES          # 4096 k columns per core
T = KS // 512             # 8 logical tiles of 4 k-chunks
WB = 2448                 # tile width in bytes (fp8 cols)
WH = 1632                 # hi+q fp16 region bytes
WL = 816                  # lo+q8 fp8 region bytes
R = 100                   # gathered rows per diagram
SIDE = 28
LOSCALE = 4096.0          # Alo pre-scale (2^12)

F8 = ml_dtypes.float8_e4m3

_cache = {}


def _build_nc():
    import concourse.bacc as bacc
    import concourse.mybir as mybir
    from concourse.tile import TileContext

    f32 = mybir.dt.float32
    f16 = mybir.dt.float16
    f8 = mybir.dt.float8e4
    nc = bacc.Bacc("TRN2", target_bir_lowering=False, debug=False,
                   num_devices=NCORES)

    # DMA pieces, 6 of them, balanced across the two HWDGE queues so the
    # per-queue FIFO completion order matches the matmul consumption order
    # (the 16 SDMA engines round-robin between queues, so each queue
    # drains at ~half the 421 GB/s aggregate while both are busy):
    #   SP/Q1:  t0 (306K) | t3t4 (613K) | t6 (306K) + the two stores
    #   ACT/Q10: t1t2 (613K) | t5 (306K) | t7 (306K)
    # Small first piece -> matmuls start early; small last pieces -> the
    # post-last-byte matmul tail and the engine-15 straggler cost stay
    # small.
    A0 = nc.declare_dram_parameter("A0", [128, WB], f8, isOutput=False)
    A1 = nc.declare_dram_parameter("A1", [128, 2 * WB], f8, isOutput=False)
    A2 = nc.declare_dram_parameter("A2", [128, 2 * WB], f8, isOutput=False)
    A3 = nc.declare_dram_parameter("A3", [128, WB], f8, isOutput=False)
    A4 = nc.declare_dram_parameter("A4", [128, WB], f8, isOutput=False)
    A5 = nc.declare_dram_parameter("A5", [128, WB], f8, isOutput=False)
    y = nc.declare_dram_parameter("y", [8, 1600], f32, isOutput=True)

    with TileContext(nc) as tc:
        with (
            tc.tile_pool(name="apool", bufs=1) as apool,
            tc.tile_pool(name="wpool", bufs=1) as wpool,
            tc.tile_pool(name="opool", bufs=1) as opool,
            tc.tile_pool(name="ps", bufs=1, space="PSUM") as pspool,
        ):
            # full-partition tiles so each lands at PSUM base partition 0
            ps_a = pspool.tile([128, 400], f32, name="ps_a", tag="ps_a")[0:8, :]
            ps_b = pspool.tile([128, 400], f32, name="ps_b", tag="ps_b")[0:8, :]
            ps3_a = pspool.tile([128, 400], f32, name="ps3_a", tag="ps3_a")[0:8, :]
            ps3_b = pspool.tile([128, 400], f32, name="ps3_b", tag="ps3_b")[0:8, :]

            # PE_HAM releases the clock gate (1.2 -> 2.4 GHz) after ~3.4 us
            # of sustained PE activity; 4 dummy matmuls (~2.5 us cold) cover
            # the PE until the first real tile lands, and the ramp completes
            # during the first real matmuls.
            warm = wpool.tile([128, 512], f16, name="warm", tag="warm")
            nc.gpsimd.memset(warm, 0.0)
            ps_w = pspool.tile([128, 512], f32, name="ps_w", tag="ps_w")[0:8, :]

            Tb0 = apool.tile([128, WB], f8, name="Tb0", tag="Tb0")
            Tb1 = apool.tile([128, 2 * WB], f8, name="Tb1", tag="Tb1")
            Tb2 = apool.tile([128, 2 * WB], f8, name="Tb2", tag="Tb2")
            Tb3 = apool.tile([128, WB], f8, name="Tb3", tag="Tb3")
            Tb4 = apool.tile([128, WB], f8, name="Tb4", tag="Tb4")
            Tb5 = apool.tile([128, WB], f8, name="Tb5", tag="Tb5")
            nc.sync.dma_start(out=Tb0, in_=A0[:])
            nc.scalar.dma_start(out=Tb1, in_=A1[:])
            nc.sync.dma_start(out=Tb2, in_=A2[:])
            nc.scalar.dma_start(out=Tb3, in_=A3[:])
            nc.sync.dma_start(out=Tb4, in_=A4[:])
            nc.scalar.dma_start(out=Tb5, in_=A5[:])

            srcs = [Tb0[:, 0:WB],
                    Tb1[:, 0:WB], Tb1[:, WB:2 * WB],
                    Tb2[:, 0:WB], Tb2[:, WB:2 * WB],
                    Tb3[:, 0:WB],
                    Tb4[:, 0:WB],
                    Tb5[:, 0:WB]]

            for _ in range(4):
                nc.tensor.matmul(ps_w, warm[:, 0:8], warm[:, 0:512],
                                 start=True, stop=True)

            # Bank parity by tile: tiles t0..t5 accumulate into the a
            # banks (stop at t5 -> their fused add and store overlap the b
            # tiles' matmuls), t6..t7 into the b banks.  Both chunk pairs
            # of a tile go to the same bank - the host sums every segment
            # partial anyway.  Intra-tile fp16/fp8 alternation keeps
            # consecutive matmuls on different PSUM banks so they pipeline.
            for t in range(T):
                src = srcs[t]
                ps, ps3 = (ps_a, ps3_a) if t < 6 else (ps_b, ps3_b)
                first = t == 0 or t == 6
                last = t == 5 or t == T - 1
                nc.tensor.matmul(
                    ps, src[:, 1600:1616].bitcast(f16),
                    src[:, 0:800].bitcast(f16), start=first, stop=False)
                nc.tensor.matmul(
                    ps3, src[:, WH + 800:WH + 808],
                    src[:, WH + 0:WH + 400], start=first, stop=False)
                nc.tensor.matmul(
                    ps, src[:, 1616:1632].bitcast(f16),
                    src[:, 800:1600].bitcast(f16), start=False, stop=last)
                nc.tensor.matmul(
                    ps3, src[:, WH + 808:WH + 816],
                    src[:, WH + 400:WH + 800], start=False, stop=last)

            # fold the fp8 term's 2^-12 scale on-device: the ACT engine
            # writes ps3/4096 while DVE copies ps; the host just adds.
            yt = opool.tile([8, 1600], f32)
            nc.vector.tensor_copy(out=yt[:, 0:400], in_=ps_a)
            nc.scalar.activation(out=yt[:, 400:800], in_=ps3_a,
                                 func=mybir.ActivationFunctionType.Copy,
                                 scale=1.0 / LOSCALE)
            nc.sync.dma_start(out=y[:, 0:800], in_=yt[:, 0:800])
            nc.vector.tensor_copy(out=yt[:, 800:1200], in_=ps_b)
            nc.scalar.activation(out=yt[:, 1200:1600], in_=ps3_b,
                                 func=mybir.ActivationFunctionType.Copy,
                                 scale=1.0 / LOSCALE)
            nc.sync.dma_start(out=y[:, 800:1600], in_=yt[:, 800:1600])
    nc.compile()
    return nc


def _split16(x):
    hi = x.astype(np.float16)
    lo = (x - hi.astype(np.float32))
    return hi, lo


def _prep_inputs(p0, p1, I1, I2, inds1, inds2):
    idx1 = inds1.astype(np.int64).reshape(-1, 2)
    idx2 = inds2.astype(np.int64).reshape(-1, 2)
    rows1 = idx1[:, 0] * SIDE + idx1[:, 1]      # flat positions, in order
    rows2 = idx2[:, 0] * SIDE + idx2[:, 1]

    selT = np.empty((K, 2 * R), np.float32)
    selT[:, 0:R] = I1[rows1, :].T
    selT[:, R:2 * R] = I2[rows2, :].T
    sel_hi, sel_lo = _split16(selT)             # fp16 [K,200], fp32 resid
    sel_lo8 = (sel_lo * LOSCALE).astype(F8)     # scaled e4m3 [K,200]
    q = np.stack([p0, p1], axis=-1)             # [K, 2]
    q_hi, _qlo = _split16(q)
    q_lo = _qlo.astype(np.float16)
    q8a = q_hi.astype(np.float32).astype(F8)
    q8b = (q_hi.astype(np.float32) - q8a.astype(np.float32)).astype(F8)

    in_maps = []
    for cix in range(NCORES):
        o = cix * KS
        bh = sel_hi[o:o + KS].reshape(T, 4, 128, 2 * R)
        bl = sel_lo8[o:o + KS].reshape(T, 4, 128, 2 * R)
        qh = q_hi[o:o + KS].reshape(T, 2, 2, 128, 2)
        ql = q_lo[o:o + KS].reshape(T, 2, 2, 128, 2)
        qa = q8a[o:o + KS].reshape(T, 2, 2, 128, 2)
        qb = q8b[o:o + KS].reshape(T, 2, 2, 128, 2)

        a16 = np.empty((T, 128, 816), np.float16)   # hi + q fp16 region
        for c in range(4):
            a16[:, :, c * 200:(c + 1) * 200] = bh[:, c]
        a16[:, :, 800:802] = qh[:, 0, 0]
        a16[:, :, 802:804] = qh[:, 0, 1]
        a16[:, :, 804:806] = ql[:, 0, 0]
        a16[:, :, 806:808] = ql[:, 0, 1]
        a16[:, :, 808:810] = qh[:, 1, 0]
        a16[:, :, 810:812] = qh[:, 1, 1]
        a16[:, :, 812:814] = ql[:, 1, 0]
        a16[:, :, 814:816] = ql[:, 1, 1]

        a8 = np.empty((T, 128, 816), F8)            # lo + q8 fp8 region
        for c in range(4):
            a8[:, :, c * 200:(c + 1) * 200] = bl[:, c]
        a8[:, :, 800:802] = qa[:, 0, 0]
        a8[:, :, 802:804] = qa[:, 0, 1]
        a8[:, :, 804:806] = qb[:, 0, 0]
        a8[:, :, 806:808] = qb[:, 0, 1]
        a8[:, :, 808:810] = qa[:, 1, 0]
        a8[:, :, 810:812] = qa[:, 1, 1]
        a8[:, :, 812:814] = qb[:, 1, 0]
        a8[:, :, 814:816] = qb[:, 1, 1]

        ab = np.empty((T, 128, WB), np.uint8)
        ab[:, :, 0:WH] = a16.view(np.uint8).reshape(T, 128, WH)
        ab[:, :, WH:WB] = a8.view(np.uint8)

        def pair(u):
            return np.ascontiguousarray(
                ab[u:u + 2].transpose(1, 0, 2)).reshape(128, 2 * WB)

        m = {"A0": ab[0].view(F8), "A1": pair(1).view(F8),
             "A2": pair(3).view(F8), "A3": ab[5].view(F8),
             "A4": ab[6].view(F8), "A5": ab[7].view(F8)}
        in_maps.append(m)
    return in_maps


def _run(in_maps, trace=False):
    from concourse.bass_utils import run_bass_kernel_spmd

    if "nc" not in _cache:
        _cache["nc"] = _build_nc()
    return run_bass_kernel_spmd(
        _cache["nc"], in_maps, list(range(NCORES)), trace=trace
    )


def kernel(p0, p1, I1, I2, inds1, inds2):
    p0 = np.ascontiguousarray(np.asarray(p0, dtype=np.float32))
    p1 = np.ascontiguousarray(np.asarray(p1, dtype=np.float32))
    I1 = np.asarray(I1, dtype=np.float32)
    I2 = np.asarray(I2, dtype=np.float32)
    inds1 = np.asarray(inds1)
    inds2 = np.asarray(inds2)

    in_maps = _prep_inputs(p0, p1, I1, I2, inds1, inds2)
    results = _run(in_maps).results

    acc = np.zeros((2, R), np.float64)
    for r in results:
        yf = r["y"].astype(np.float64)
        for base in (0, 400, 800, 1200):
            yc = yf[:, base:base + 400]
            acc[0] += (yc[0, 0:100] + yc[4, 0:100]
                       + yc[2, 200:300] + yc[6, 200:300])      # dgm1
            acc[1] += (yc[1, 100:200] + yc[5, 100:200]
                       + yc[3, 300:400] + yc[7, 300:400])      # dgm2
    vals = acc.astype(np.float32)
    dgm1 = vals[0].reshape(R // 2, 2)
    dgm2 = vals[1].reshape(R // 2, 2)
    return (dgm1, dgm2)
